# revision 8
# baseline (speedup 1.0000x reference)
"""ConvTreeGRUCell on 8 Trainium2 NeuronCores.

Sharding: spatial over H. Each core owns 24 output rows (192/8) and
receives a 28-row input slab (2-row halo each side, zero-padded at the
image borders on the host). All three 3x3 convs, the per-child reset
gate, and the L-reductions are then fully local per core — no
collectives.

Per-core kernel (Tile framework):
  - frames are 28 rows x 194 cols (192 + zero pad col each side),
    flattened to 5432 elements + 1 front/back pad -> free size 5434.
    A 3x3 conv tap (dy,dx) is a single offset dy*194+dx into the flat
    frame; the zero pad columns absorb the row-wrap reads.
  - cat tiles [128, 5434]: partitions 0..63 = child_h[l] channels,
    64..127 = x channels (child first so every elementwise op on
    child/r/reset_hidden/child_sum shares partition base 0 — the
    walrus verifier requires identical partition ranges).  conv = 9 accumulating fp32r matmuls
    (K=128, M=64, N<=512 pixel windows) into one PSUM bank.
  - r_l = sigmoid(psum + br) on ScalarE (bias is per-partition).
  - reset_hidden accumulated on VectorE; child_h_sum on GPSIMD.
  - z/o convs run over [x | child_sum] and [x | reset_hidden] cat
    tiles; h = o + z*(child_sum - o).
"""

import os
import sys

import numpy as np

for _p in ("/opt/trn_rl_repo",):
    if _p not in sys.path and os.path.isdir(_p):
        sys.path.insert(0, _p)

import concourse.bass as bass
import concourse.tile as tile
from concourse import bacc
from concourse import mybir
from concourse.bass_utils import run_bass_kernel_spmd

F32 = mybir.dt.float32
F32R = mybir.dt.float32r

C = 64          # channels
L = 8           # children
HW = 192        # image H and W
NCORES = 8
OUT_ROWS = HW // NCORES          # 24 output rows per core
IN_ROWS = OUT_ROWS + 4           # 28-row slab (2-row halo each side)
WP = HW + 2                      # 194: padded row width
FRAME = IN_ROWS * WP             # 5432
FREE = FRAME + 2                 # 5434: +1 front pad, +1 tail pad
HALF = FREE // 2                 # 2717

# flat index of (row r, col c) in the frame = 1 + r*WP + c
# stage 1 (r gate / reset_hidden): output rows 1..26
S1_LO = 1 + 1 * WP               # 195
S1_HI = 1 + 26 * WP + 194       # 5239 (exclusive)
# stage 2 (z / o / h): output rows 2..25
S2_LO = 1 + 2 * WP               # 389
S2_HI = 1 + 25 * WP + 194       # 5045 (exclusive)

NWIN = 512

TAP_OFF = [dy * WP + dx for dy in (-1, 0, 1) for dx in (-1, 0, 1)]


def _windows(lo, hi):
    out = []
    s = lo
    while s < hi:
        out.append((s, min(NWIN, hi - s)))
        s += NWIN
    return out


S1WIN = _windows(S1_LO, S1_HI)
S2WIN = _windows(S2_LO, S2_HI)

_BUILT = None


def build_program():
    """Build the (SPMD, per-core) Bass program once."""
    nc = bacc.Bacc("TRN2")

    xin = nc.dram_tensor("xin", [C, FREE], F32, kind="ExternalInput")
    cin = nc.dram_tensor("cin", [L, C, FREE], F32, kind="ExternalInput")
    wrt = nc.dram_tensor("wrt", [2 * C, 9, C], F32, kind="ExternalInput")
    wzt = nc.dram_tensor("wzt", [2 * C, 9, C], F32, kind="ExternalInput")
    wot = nc.dram_tensor("wot", [2 * C, 9, C], F32, kind="ExternalInput")
    brt = nc.dram_tensor("brt", [C, 1], F32, kind="ExternalInput")
    bzt = nc.dram_tensor("bzt", [C, 1], F32, kind="ExternalInput")
    bot = nc.dram_tensor("bot", [C, 1], F32, kind="ExternalInput")
    hout = nc.dram_tensor("hout", [C, OUT_ROWS, HW], F32, kind="ExternalOutput")

    with tile.TileContext(nc) as tc:
        with (
            tc.tile_pool(name="singles", bufs=1) as singles,
            tc.tile_pool(name="cats", bufs=3) as cats,
            tc.tile_pool(name="rwin", bufs=4) as rwin_pool,
            tc.tile_pool(name="twin", bufs=4) as twin_pool,
            tc.tile_pool(name="s2", bufs=4) as s2_pool,
            tc.tile_pool(name="psum", bufs=6, space="PSUM") as psum_pool,
        ):
            # ---- persistent tiles ----
            xsrc = singles.tile([2 * C, HALF], F32R, tag="xsrc")
            wr = singles.tile([2 * C, 9, C], F32R, tag="wr")
            wz = singles.tile([2 * C, 9, C], F32R, tag="wz")
            wo = singles.tile([2 * C, 9, C], F32R, tag="wo")
            br = singles.tile([C, 1], F32, tag="br")
            bz = singles.tile([C, 1], F32, tag="bz")
            bo = singles.tile([C, 1], F32, tag="bo")
            zs = singles.tile([2 * C, FREE], F32R, tag="zs")    # [x | child_sum]
            orh = singles.tile([2 * C, FREE], F32R, tag="orh")  # [x | reset_hidden]
            ht = singles.tile([C, OUT_ROWS * WP], F32, tag="ht")

            # ---- loads ----
            # x frame halves stacked on partitions: (half, channel)
            nc.gpsimd.dma_start(out=xsrc[0:C, :], in_=xin[:, 0:HALF])
            nc.gpsimd.dma_start(out=xsrc[C:2 * C, :], in_=xin[:, HALF:FREE])
            nc.gpsimd.dma_start(out=wr, in_=wrt[:])
            nc.gpsimd.dma_start(out=wz, in_=wzt[:])
            nc.gpsimd.dma_start(out=wo, in_=wot[:])
            nc.sync.dma_start(out=br, in_=brt[:])
            nc.sync.dma_start(out=bz, in_=bzt[:])
            nc.sync.dma_start(out=bo, in_=bot[:])

            # x halves into the stage-2 cat tiles
            for dst in (zs, orh):
                nc.sync.dma_start(out=dst[C:2 * C, 0:HALF], in_=xsrc[0:C, :])
                nc.sync.dma_start(out=dst[C:2 * C, HALF:FREE], in_=xsrc[C:2 * C, :])
            # reset_hidden rows 0 and 27 (and the flat pads) are never
            # written by the accumulation below but are read by the o-conv.
            # memset can't produce f32r; bounce zeros through a f32 scratch.
            zpad = singles.tile([C, S1_LO], F32, tag="zpad")
            nc.vector.memset(zpad, 0.0)
            nc.vector.tensor_copy(out=orh[0:C, 0:S1_LO], in_=zpad)
            nc.vector.tensor_copy(out=orh[0:C, S1_HI:FREE], in_=zpad[:, 0:FREE - S1_HI])

            # ---- stage 1: per-child reset gate + reductions ----
            for l in range(L):
                cat = cats.tile([2 * C, FREE], F32R, tag="cat")
                nc.gpsimd.dma_start(out=cat[0:C, :], in_=cin[l])
                nc.sync.dma_start(out=cat[C:2 * C, 0:HALF], in_=xsrc[0:C, :])
                nc.sync.dma_start(out=cat[C:2 * C, HALF:FREE], in_=xsrc[C:2 * C, :])

                # child_h_sum accumulation on GPSIMD
                if l == 0:
                    nc.gpsimd.tensor_copy(
                        out=zs[0:C, :], in_=cat[0:C, :].bitcast(F32)
                    )
                else:
                    nc.gpsimd.tensor_add(
                        out=zs[0:C, :],
                        in0=zs[0:C, :].bitcast(F32),
                        in1=cat[0:C, :].bitcast(F32),
                    )

                for s, n in S1WIN:
                    ps = psum_pool.tile([C, NWIN], F32, tag="ps")
                    for t in range(9):
                        o = TAP_OFF[t]
                        nc.tensor.matmul(
                            out=ps[:, :n],
                            lhsT=wr[:, t, :],
                            rhs=cat[:, s + o:s + o + n],
                            start=(t == 0),
                            stop=(t == 8),
                        )
                    rw = rwin_pool.tile([C, NWIN], F32, tag="rw")
                    nc.scalar.activation(
                        out=rw[:, :n],
                        in_=ps[:, :n],
                        func=mybir.ActivationFunctionType.Sigmoid,
                        bias=br[:, 0:1],
                    )
                    if l == 0:
                        nc.vector.tensor_mul(
                            out=orh[0:C, s:s + n],
                            in0=rw[:, :n],
                            in1=cat[0:C, s:s + n].bitcast(F32),
                        )
                    else:
                        tw = twin_pool.tile([C, NWIN], F32, tag="tw")
                        nc.vector.tensor_mul(
                            out=tw[:, :n],
                            in0=rw[:, :n],
                            in1=cat[0:C, s:s + n].bitcast(F32),
                        )
                        nc.vector.tensor_add(
                            out=orh[0:C, s:s + n],
                            in0=orh[0:C, s:s + n].bitcast(F32),
                            in1=tw[:, :n],
                        )

            # ---- stage 2: z, o, h ----
            for wi, (s, n) in enumerate(S2WIN):
                psz = psum_pool.tile([C, NWIN], F32, tag="ps")
                for t in range(9):
                    o = TAP_OFF[t]
                    nc.tensor.matmul(
                        out=psz[:, :n],
                        lhsT=wz[:, t, :],
                        rhs=zs[:, s + o:s + o + n],
                        start=(t == 0),
                        stop=(t == 8),
                    )
                zw = s2_pool.tile([C, NWIN], F32, tag="zw")
                nc.scalar.activation(
                    out=zw[:, :n],
                    in_=psz[:, :n],
                    func=mybir.ActivationFunctionType.Sigmoid,
                    bias=bz[:, 0:1],
                )

                pso = psum_pool.tile([C, NWIN], F32, tag="ps")
                for t in range(9):
                    o = TAP_OFF[t]
                    nc.tensor.matmul(
                        out=pso[:, :n],
                        lhsT=wo[:, t, :],
                        rhs=orh[:, s + o:s + o + n],
                        start=(t == 0),
                        stop=(t == 8),
                    )
                ow = s2_pool.tile([C, NWIN], F32, tag="ow")
                nc.scalar.activation(
                    out=ow[:, :n],
                    in_=pso[:, :n],
                    func=mybir.ActivationFunctionType.Tanh,
                    bias=bo[:, 0:1],
                )

                # h = o + z * (child_sum - o)
                t1 = s2_pool.tile([C, NWIN], F32, tag="t1")
                nc.vector.scalar_tensor_tensor(
                    out=t1[:, :n],
                    in0=ow[:, :n],
                    scalar=-1.0,
                    in1=zs[0:C, s:s + n].bitcast(F32),
                    op0=mybir.AluOpType.mult,
                    op1=mybir.AluOpType.add,
                )
                t2 = s2_pool.tile([C, NWIN], F32, tag="t2")
                nc.vector.tensor_mul(out=t2[:, :n], in0=zw[:, :n], in1=t1[:, :n])
                j = s - S2_LO
                nc.vector.tensor_add(
                    out=ht[:, j:j + n], in0=ow[:, :n], in1=t2[:, :n]
                )

            # ---- store: drop the pad column of each row ----
            htr = ht.rearrange("p (r w) -> p r w", w=WP)
            nc.sync.dma_start(out=hout[:], in_=htr[:, 0:OUT_ROWS, 1:HW + 1])

    nc.finalize()
    return nc


def _get_program():
    global _BUILT
    if _BUILT is None:
        _BUILT = build_program()
    return _BUILT


def make_in_maps(x, child_h, Wr, br, Wz, bz, Wo, bo):
    """Host-side sharding: pad borders/columns and slice 28-row slabs."""
    x = np.asarray(x, dtype=np.float32)
    child_h = np.asarray(child_h, dtype=np.float32)

    # zero-pad H by 2 (halo at image border) and W by 1 (conv column pad)
    xp = np.zeros((C, HW + 4, WP), dtype=np.float32)
    xp[:, 2:2 + HW, 1:1 + HW] = x[0]
    cp = np.zeros((L, C, HW + 4, WP), dtype=np.float32)
    cp[:, :, 2:2 + HW, 1:1 + HW] = child_h[:, 0]

    def frame(a):  # [..., IN_ROWS, WP] -> [..., FREE] with 1-elem front/tail pad
        flat = a.reshape(a.shape[:-2] + (FRAME,))
        out = np.zeros(a.shape[:-2] + (FREE,), dtype=np.float32)
        out[..., 1:1 + FRAME] = flat
        return out

    def wprep(w):  # [C, 2C, 3, 3] -> [2C, 9, C] lhsT per tap
        wt = np.transpose(np.asarray(w, np.float32), (1, 2, 3, 0)).reshape(2 * C, 9, C)
        # cat layout is [child | x], reference weight rows are [x | child]
        return np.ascontiguousarray(np.concatenate([wt[C:], wt[:C]], axis=0))

    wrt, wzt, wot = wprep(Wr), wprep(Wz), wprep(Wo)
    brt = np.asarray(br, np.float32).reshape(C, 1)
    bzt = np.asarray(bz, np.float32).reshape(C, 1)
    bot = np.asarray(bo, np.float32).reshape(C, 1)

    in_maps = []
    for k in range(NCORES):
        r0 = k * OUT_ROWS  # global output row start; slab = rows r0-2 .. r0+26
        in_maps.append({
            "xin": frame(xp[:, r0:r0 + IN_ROWS, :]),
            "cin": frame(cp[:, :, r0:r0 + IN_ROWS, :]),
            "wrt": wrt, "wzt": wzt, "wot": wot,
            "brt": brt, "bzt": bzt, "bot": bot,
        })
    return in_maps


def run(in_maps, trace=False):
    nc = _get_program()
    return run_bass_kernel_spmd(nc, in_maps, list(range(NCORES)), trace=trace)


def kernel(x, child_h, Wr, br, Wz, bz, Wo, bo):
    in_maps = make_in_maps(x, child_h, Wr, br, Wz, bz, Wo, bo)
    res = run(in_maps).results
    out = np.empty((1, C, HW, HW), dtype=np.float32)
    for k in range(NCORES):
        out[0, :, k * OUT_ROWS:(k + 1) * OUT_ROWS, :] = res[k]["hout"]
    return out
